# revision 2
# baseline (speedup 1.0000x reference)
"""Bahdanau attention kernel for 8 Trainium2 NeuronCores.

Problem (hardcoded shapes): B=32, T=8192, D_ENC=256, D_HID=512, D_ATT=512.
    proj = encoder_out @ w1 + b1 + (h @ w2 + b2) + (c @ w3 + b3)   # [B,T,512]
    scores = tanh(proj) @ wv (+ bv)                                # [B,T,1]
    attn = softmax(scores, axis=T)
    context = sum_t attn * encoder_out                             # [B,256]

Sharding: data-parallel over batch, 4 batches per core, no collectives.

Device strategy (per core, per batch):
  - encoder_out is fed twice in bf16 (transposed [256,8192] for the
    projection matmul; natural [8192,256] for the context matmul) — the two
    bf16 copies cost the same HBM traffic as one f32 copy.
  - Pass A (16 chunks of 512 timesteps): hidden^T[j] = w1[k,j]^T @ encT
    accumulated over k in PSUM; tanh with the per-batch bias fused as the
    ACT per-partition bias; scores = sum_j wv_j^T @ tanh_j (PE, M=1);
    scores are transposed to column form via PE transpose into a per-batch
    [128, 64] PSUM tile.
  - exp over the whole batch's scores in one ACT instruction (scores are
    O(1) so no max subtraction is needed; a constant shift cancels in
    softmax anyway, which is also why bv is dropped).
  - Pass B (64 chunks of 128 timesteps): ctx += e_col^T @ enc_nat on PE,
    Z = sum(e); finally context = ctx / Z.
  Pass B of batch b is emitted interleaved with pass A of batch b+1 so PE
  and DMA stay busy.
"""

import os
import sys

for _p in ("/opt/trn_rl_repo", "/root/.axon_site", "/root/.axon_site/_ro/pypackages"):
    if os.path.isdir(_p) and _p not in sys.path:
        sys.path.append(_p)

import numpy as np
import ml_dtypes

import concourse.bass as bass
import concourse.tile as tile
from concourse import bacc, mybir
from concourse.bass_utils import run_bass_kernel_spmd

BF16 = ml_dtypes.bfloat16

B, T, D_ENC, D_HID, D_ATT = 32, 8192, 256, 512, 512
N_CORES = 8
BPC = B // N_CORES          # batches per core = 4
P = 128                     # partitions
TC = 512                    # pass-A chunk (timesteps)
NCH = T // TC               # pass-A chunks per batch = 16
NCOL = T // P               # score columns per batch = 64
KD = D_ENC // P             # k-tiles of the contraction dim = 2
NJ = D_ATT // P             # a-tiles = 4

_PROGRAM_CACHE = {}


def _build_program():
    """Build and finalize the SPMD program (identical on all 8 cores)."""
    if "nc" in _PROGRAM_CACHE:
        return _PROGRAM_CACHE["nc"]

    f32 = mybir.dt.float32
    bf16 = mybir.dt.bfloat16
    Act = mybir.ActivationFunctionType

    nc = bacc.Bacc("TRN2", target_bir_lowering=False, debug=False,
                   num_devices=N_CORES)

    encT = nc.dram_tensor("encT", [BPC, D_ENC, T], bf16, kind="ExternalInput")
    encN = nc.dram_tensor("encN", [BPC, T, D_ENC], bf16, kind="ExternalInput")
    w1t = nc.dram_tensor("w1t", [P, KD, NJ, P], bf16, kind="ExternalInput")
    wvt = nc.dram_tensor("wvt", [P, NJ], bf16, kind="ExternalInput")
    vbt = nc.dram_tensor("vbt", [P, BPC * NJ], f32, kind="ExternalInput")
    outd = nc.dram_tensor("out", [BPC, D_ENC], f32, kind="ExternalOutput")

    with tile.TileContext(nc) as tc:
        import contextlib
        with contextlib.ExitStack() as ctx:
            const = ctx.enter_context(tc.tile_pool(name="const", bufs=1))
            encT_pool = ctx.enter_context(tc.tile_pool(name="encT", bufs=4))
            encN_pool = ctx.enter_context(tc.tile_pool(name="encN", bufs=4))
            tanh_pool = ctx.enter_context(tc.tile_pool(name="tanh", bufs=8))
            ssb_pool = ctx.enter_context(tc.tile_pool(name="ssb", bufs=3))
            e_pool = ctx.enter_context(tc.tile_pool(name="e", bufs=2))
            sm_pool = ctx.enter_context(tc.tile_pool(name="sm", bufs=4))
            osb_pool = ctx.enter_context(tc.tile_pool(name="osb", bufs=2))
            hid_psum = ctx.enter_context(
                tc.tile_pool(name="hid", bufs=4, space="PSUM"))
            sc_psum = ctx.enter_context(
                tc.tile_pool(name="sc", bufs=1, space="PSUM"))
            epre_psum = ctx.enter_context(
                tc.tile_pool(name="epre", bufs=2, space="PSUM"))
            ctx_psum = ctx.enter_context(
                tc.tile_pool(name="ctxz", bufs=1, space="PSUM"))

            # constants
            w1_sb = const.tile([P, KD, NJ, P], bf16)
            nc.sync.dma_start(w1_sb[:], w1t[:])
            wvt_sb = const.tile([P, NJ], bf16)
            nc.sync.dma_start(wvt_sb[:], wvt[:])
            vbt_sb = const.tile([P, BPC * NJ], f32)
            nc.sync.dma_start(vbt_sb[:], vbt[:])
            ones128 = const.tile([P, 1], f32)
            nc.gpsimd.memset(ones128[:], 1.0)
            ones11 = const.tile([1, 1], f32)
            nc.gpsimd.memset(ones11[:], 1.0)

            epre = {}    # per-batch [128, NCOL] psum: scores in column form
            e_sb = {}    # per-batch [128, NCOL] bf16: exp(scores)
            ctxz = {}    # per-batch [1, 257] psum: ctx accum + Z
            rz = {}      # per-batch [1, 1] f32: 1/Z

            def emit_A_chunk(b, i):
                encT_t = encT_pool.tile([P, KD, TC], bf16)
                nc.sync.dma_start(
                    encT_t[:],
                    encT[b, :, i * TC:(i + 1) * TC]
                        .rearrange("(k p) t -> p k t", p=P))
                tanh_tiles = []
                for j in range(NJ):
                    h_ps = hid_psum.tile([P, TC], f32, tag="hid")
                    for k in range(KD):
                        nc.tensor.matmul(h_ps[:], w1_sb[:, k, j, :],
                                         encT_t[:, k, :],
                                         start=(k == 0), stop=(k == KD - 1))
                    th = tanh_pool.tile([P, TC], bf16, tag="tanh")
                    nc.scalar.activation(
                        th[:], h_ps[:], Act.Tanh,
                        bias=vbt_sb[:, b * NJ + j: b * NJ + j + 1])
                    tanh_tiles.append(th)
                s_ps = sc_psum.tile([1, TC], f32, tag="sc")
                for j in range(NJ):
                    nc.tensor.matmul(s_ps[:], wvt_sb[:, j:j + 1],
                                     tanh_tiles[j][:],
                                     start=(j == 0), stop=(j == NJ - 1))
                s_sb = ssb_pool.tile([1, TC], f32, tag="ssb")
                nc.vector.tensor_copy(s_sb[:], s_ps[:])
                for u in range(TC // P):
                    col = i * (TC // P) + u
                    nc.tensor.transpose(epre[b][:, col:col + 1],
                                        s_sb[:, u * P:(u + 1) * P], ones11[:])

            def emit_A_epilogue(b):
                e_sb[b] = e_pool.tile([P, NCOL], bf16, tag="e", name=f"e_sb{b}")
                nc.scalar.activation(e_sb[b][:], epre[b][:], Act.Exp)
                z128 = sm_pool.tile([P, 1], f32, tag="z128")
                nc.vector.reduce_sum(z128[:], e_sb[b][:],
                                     axis=mybir.AxisListType.X)
                ctxz[b] = ctx_psum.tile([1, D_ENC + 1], f32, tag="ctxz", name=f"ctxz{b}")
                nc.tensor.matmul(ctxz[b][:, D_ENC:D_ENC + 1], z128[:],
                                 ones128[:])
                rz[b] = sm_pool.tile([1, 1], f32, tag="rz", name=f"rz{b}")
                nc.vector.reciprocal(rz[b][:], ctxz[b][:, D_ENC:D_ENC + 1])

            def emit_B_group(b, g):
                encN_t = encN_pool.tile([P, TC // P, D_ENC], bf16)
                nc.sync.dma_start(
                    encN_t[:],
                    encN[b, g * TC:(g + 1) * TC, :]
                        .rearrange("(n p) d -> p n d", p=P))
                for n in range(TC // P):
                    m = (TC // P) * g + n
                    nc.tensor.matmul(ctxz[b][:, 0:D_ENC],
                                     e_sb[b][:, m:m + 1], encN_t[:, n, :],
                                     start=(m == 0), stop=(m == NCOL - 1))

            def emit_B_finalize(b):
                o_sb = osb_pool.tile([1, D_ENC], f32, tag="osb")
                nc.vector.tensor_scalar_mul(o_sb[:], ctxz[b][:, 0:D_ENC],
                                            rz[b][:])
                nc.sync.dma_start(outd[b:b + 1, :], o_sb[:])

            for step in range(BPC + 1):
                if step < BPC:
                    epre[step] = epre_psum.tile([P, NCOL], f32, tag="epre", name=f"epre{step}")
                for i in range(NCH):
                    if step < BPC:
                        emit_A_chunk(step, i)
                    if step >= 1:
                        emit_B_group(step - 1, i)
                if step < BPC:
                    emit_A_epilogue(step)
                if step >= 1:
                    emit_B_finalize(step - 1)

    nc.finalize()
    _PROGRAM_CACHE["nc"] = nc
    return nc


def _prep_inputs(encoder_out, hidden_state_h, hidden_state_c,
                 w1, b1, w2, b2, w3, b3, wv, bv):
    """Host-side sharding + layout prep. Returns per-core input maps."""
    enc = np.asarray(encoder_out, dtype=np.float32)
    # per-batch bias vector: b1 + h@w2 + b2 + c@w3 + b3  (tiny, exact f32)
    vb = (np.asarray(b1, np.float32)
          + np.asarray(hidden_state_h, np.float32) @ np.asarray(w2, np.float32)
          + np.asarray(b2, np.float32)
          + np.asarray(hidden_state_c, np.float32) @ np.asarray(w3, np.float32)
          + np.asarray(b3, np.float32))                        # [B, D_ATT]
    # bv shifts every score equally -> cancels in softmax; dropped.

    w1_h = np.ascontiguousarray(
        np.asarray(w1, np.float32).reshape(KD, P, NJ, P).transpose(1, 0, 2, 3)
    ).astype(BF16)                                             # [128,2,4,128]
    wv_h = np.ascontiguousarray(
        np.asarray(wv, np.float32).reshape(NJ, P).T).astype(BF16)  # [128,4]

    in_maps = []
    for c in range(N_CORES):
        sl = slice(c * BPC, (c + 1) * BPC)
        enc_c = enc[sl]                                        # [4, T, 256]
        encT_c = np.ascontiguousarray(enc_c.transpose(0, 2, 1)).astype(BF16)
        encN_c = np.ascontiguousarray(enc_c).astype(BF16)
        vbt_c = np.ascontiguousarray(
            vb[sl].reshape(BPC, NJ, P).transpose(2, 0, 1).reshape(P, BPC * NJ)
        ).astype(np.float32)
        in_maps.append({
            "encT": encT_c,
            "encN": encN_c,
            "w1t": w1_h,
            "wvt": wv_h,
            "vbt": vbt_c,
        })
    return in_maps


def kernel(**inputs):
    nc = _build_program()
    in_maps = _prep_inputs(**inputs)
    res = run_bass_kernel_spmd(nc, in_maps, list(range(N_CORES)))
    out = np.concatenate([res.results[c]["out"] for c in range(N_CORES)],
                         axis=0)
    return out.astype(np.float32)


if __name__ == "__main__":
    rng = np.random.default_rng(0)
    ins = {
        "encoder_out": rng.standard_normal((B, T, D_ENC), dtype=np.float32),
        "hidden_state_h": rng.standard_normal((B, D_HID), dtype=np.float32),
        "hidden_state_c": rng.standard_normal((B, D_HID), dtype=np.float32),
        "w1": (rng.standard_normal((D_ENC, D_ATT), dtype=np.float32)
               / np.sqrt(D_ENC)),
        "b1": np.zeros(D_ATT, np.float32),
        "w2": (rng.standard_normal((D_HID, D_ATT), dtype=np.float32)
               / np.sqrt(D_HID)),
        "b2": np.zeros(D_ATT, np.float32),
        "w3": (rng.standard_normal((D_HID, D_ATT), dtype=np.float32)
               / np.sqrt(D_HID)),
        "b3": np.zeros(D_ATT, np.float32),
        "wv": (rng.standard_normal((D_ATT, 1), dtype=np.float32)
               / np.sqrt(D_ATT)),
        "bv": np.zeros(1, np.float32),
    }
    got = kernel(**ins)
    print("kernel output:", got.shape, got.dtype)


# revision 5
# speedup vs baseline: 1.0891x; 1.0891x over previous
"""Bahdanau attention kernel for 8 Trainium2 NeuronCores.

Problem (hardcoded shapes): B=32, T=8192, D_ENC=256, D_HID=512, D_ATT=512.
    proj = encoder_out @ w1 + b1 + (h @ w2 + b2) + (c @ w3 + b3)   # [B,T,512]
    scores = tanh(proj) @ wv (+ bv)                                # [B,T,1]
    attn = softmax(scores, axis=T)
    context = sum_t attn * encoder_out                             # [B,256]

Sharding: data-parallel over batch, 4 batches per core, no collectives.

Device strategy (per core, per batch):
  - encoder_out is fed twice in bf16 (transposed [256,8192] for the
    projection matmul; natural [8192,256] for the context matmul) — the two
    bf16 copies cost the same HBM traffic as one f32 copy.
  - Pass A (8 chunks of 1024 timesteps): hidden^T[j] = w1[k,j]^T @ encT
    accumulated over k in PSUM; tanh with the per-batch bias fused as the
    ACT per-partition bias (one FD=1024 instruction per j);
    scores = sum_j wv_j^T @ tanh_j on PE (M=1); score rows are collected
    into a per-batch [8, 1024] SBUF tile via DVE copies.
  - Per batch: 8 PE transposes ([8,128] blocks against a host-fed identity)
    turn the score rows into column form [128, 8, 8] in PSUM, then one ACT
    exp produces e in bf16.  Scores are O(1) so no max subtraction is
    needed; a constant shift cancels in softmax anyway, which is also why
    bv is dropped.
  - Pass B (64 chunks of 128 timesteps): ctx += e_col^T @ enc_nat on PE,
    Z = sum(e); finally context = ctx / Z.
  Pass B of batch b is emitted interleaved with pass A of batch b+1 so PE
  and DMA stay busy.
"""

import os
import sys

for _p in ("/opt/trn_rl_repo", "/root/.axon_site", "/root/.axon_site/_ro/pypackages"):
    if os.path.isdir(_p) and _p not in sys.path:
        sys.path.append(_p)

import numpy as np
import ml_dtypes

import concourse.bass as bass
import concourse.tile as tile
from concourse import bacc, mybir
from concourse.bass_utils import run_bass_kernel_spmd

BF16 = ml_dtypes.bfloat16

B, T, D_ENC, D_HID, D_ATT = 32, 8192, 256, 512, 512
N_CORES = 8
BPC = B // N_CORES          # batches per core = 4
P = 128                     # partitions
TC = 1024                   # pass-A chunk (timesteps)
HTC = TC // 2               # matmul moving-dim half = 512
NCH = T // TC               # pass-A chunks per batch = 8
NU = TC // P                # 128-blocks per chunk = 8
NCOL = T // P               # score columns per batch = 64
KD = D_ENC // P             # k-tiles of the contraction dim = 2
NJ = D_ATT // P             # a-tiles = 4
GB = 512                    # pass-B DMA group (timesteps)
NGB = T // GB               # pass-B groups per batch = 16

_PROGRAM_CACHE = {}


def _build_program():
    """Build and finalize the SPMD program (identical on all 8 cores)."""
    if "nc" in _PROGRAM_CACHE:
        return _PROGRAM_CACHE["nc"]

    f32 = mybir.dt.float32
    bf16 = mybir.dt.bfloat16
    Act = mybir.ActivationFunctionType

    nc = bacc.Bacc("TRN2", target_bir_lowering=False, debug=False,
                   num_devices=N_CORES)

    encT = nc.dram_tensor("encT", [BPC, D_ENC, T], bf16, kind="ExternalInput")
    encN = nc.dram_tensor("encN", [BPC, T, D_ENC], bf16, kind="ExternalInput")
    w1t = nc.dram_tensor("w1t", [P, KD, NJ, P], bf16, kind="ExternalInput")
    wvt = nc.dram_tensor("wvt", [P, NJ], bf16, kind="ExternalInput")
    vbt = nc.dram_tensor("vbt", [P, BPC * NJ], f32, kind="ExternalInput")
    ident = nc.dram_tensor("ident", [NCH, NCH], f32, kind="ExternalInput")
    outd = nc.dram_tensor("out", [BPC, D_ENC], f32, kind="ExternalOutput")

    with tile.TileContext(nc) as tc:
        import contextlib
        with contextlib.ExitStack() as ctx:
            const = ctx.enter_context(tc.tile_pool(name="const", bufs=1))
            encT_pool = ctx.enter_context(tc.tile_pool(name="encT", bufs=3))
            encN_pool = ctx.enter_context(tc.tile_pool(name="encN", bufs=4))
            tanh_pool = ctx.enter_context(tc.tile_pool(name="tanh", bufs=8))
            s_pool = ctx.enter_context(tc.tile_pool(name="srows", bufs=2))
            ssb_pool = ctx.enter_context(tc.tile_pool(name="ssb", bufs=3))
            e_pool = ctx.enter_context(tc.tile_pool(name="e", bufs=2))
            sm_pool = ctx.enter_context(tc.tile_pool(name="sm", bufs=4))
            osb_pool = ctx.enter_context(tc.tile_pool(name="osb", bufs=2))
            hid_psum = ctx.enter_context(
                tc.tile_pool(name="hid", bufs=2, space="PSUM"))
            sc_psum = ctx.enter_context(
                tc.tile_pool(name="sc", bufs=1, space="PSUM"))
            epre_psum = ctx.enter_context(
                tc.tile_pool(name="epre", bufs=1, space="PSUM"))
            ctx_psum = ctx.enter_context(
                tc.tile_pool(name="ctxz", bufs=1, space="PSUM"))

            # constants
            w1_sb = const.tile([P, KD, NJ, P], bf16)
            nc.sync.dma_start(w1_sb[:], w1t[:])
            wvt_sb = const.tile([P, NJ], bf16)
            nc.sync.dma_start(wvt_sb[:], wvt[:])
            vbt_sb = const.tile([P, BPC * NJ], f32)
            nc.sync.dma_start(vbt_sb[:], vbt[:])
            ident_sb = const.tile([NCH, NCH], f32)
            nc.sync.dma_start(ident_sb[:], ident[:])
            ones128 = const.tile([P, 1], f32)
            nc.gpsimd.memset(ones128[:], 1.0)

            S = {}       # per-batch [8, 1024] f32: score rows
            epre = {}    # per-batch [128, 8, 8] psum: scores, column form
            e_sb = {}    # per-batch [128, 8, 8] bf16: exp(scores)
            ctxz = {}    # per-batch [1, 257] psum: ctx accum + Z
            rz = {}      # per-batch [1, 1] f32: 1/Z

            def emit_A_chunk(b, i):
                encT_t = encT_pool.tile([P, KD, TC], bf16)
                nc.sync.dma_start(
                    encT_t[:],
                    encT[b, :, i * TC:(i + 1) * TC]
                        .rearrange("(k p) t -> p k t", p=P))
                tanh_tiles = []
                for j in range(NJ):
                    h_ps = hid_psum.tile([P, TC], f32, tag="hid")
                    for k in range(KD):
                        for h in range(2):
                            nc.tensor.matmul(
                                h_ps[:, h * HTC:(h + 1) * HTC],
                                w1_sb[:, k, j, :],
                                encT_t[:, k, h * HTC:(h + 1) * HTC],
                                start=(k == 0), stop=(k == KD - 1))
                    th = tanh_pool.tile([P, TC], bf16, tag="tanh")
                    nc.scalar.activation(
                        th[:], h_ps[:], Act.Tanh,
                        bias=vbt_sb[:, b * NJ + j: b * NJ + j + 1])
                    tanh_tiles.append(th)
                s_ps = sc_psum.tile([1, TC], f32, tag="sc")
                for h in range(2):
                    for j in range(NJ):
                        nc.tensor.matmul(
                            s_ps[:, h * HTC:(h + 1) * HTC],
                            wvt_sb[:, j:j + 1],
                            tanh_tiles[j][:, h * HTC:(h + 1) * HTC],
                            start=(j == 0), stop=(j == NJ - 1))
                # Engine APs must start at a 32-aligned partition, so the
                # score row is staged at partition 0 and a tiny contiguous
                # SBUF->SBUF DMA drops it into row i of the S tile.
                s_sb = ssb_pool.tile([1, TC], f32, tag="ssb")
                nc.vector.tensor_copy(s_sb[:], s_ps[:])
                nc.sync.dma_start(S[b][i:i + 1, :], s_sb[:])

            def emit_A_epilogue(b):
                epre[b] = epre_psum.tile([P, NCH, NU], f32, tag="epre",
                                         name=f"epre{b}")
                for u in range(NU):
                    nc.tensor.transpose(epre[b][:, :, u],
                                        S[b][:, u * P:(u + 1) * P],
                                        ident_sb[:])
                e_sb[b] = e_pool.tile([P, NCH, NU], bf16, tag="e",
                                      name=f"e_sb{b}")
                nc.scalar.activation(e_sb[b][:], epre[b][:], Act.Exp)
                z128 = sm_pool.tile([P, 1], f32, tag="z128")
                nc.vector.reduce_sum(z128[:], e_sb[b][:],
                                     axis=mybir.AxisListType.XY)
                ctxz[b] = ctx_psum.tile([1, D_ENC + 1], f32, tag="ctxz",
                                        name=f"ctxz{b}")
                nc.tensor.matmul(ctxz[b][:, D_ENC:D_ENC + 1], z128[:],
                                 ones128[:])
                rz[b] = sm_pool.tile([1, 1], f32, tag="rz", name=f"rz{b}")
                nc.vector.reciprocal(rz[b][:], ctxz[b][:, D_ENC:D_ENC + 1])

            def emit_B_group(b, g):
                encN_t = encN_pool.tile([P, GB // P, D_ENC], bf16)
                nc.sync.dma_start(
                    encN_t[:],
                    encN[b, g * GB:(g + 1) * GB, :]
                        .rearrange("(n p) d -> p n d", p=P))
                for n in range(GB // P):
                    m = (GB // P) * g + n
                    nc.tensor.matmul(ctxz[b][:, 0:D_ENC],
                                     e_sb[b][:, m // NU, m % NU: m % NU + 1],
                                     encN_t[:, n, :],
                                     start=(m == 0), stop=(m == NCOL - 1))

            def emit_B_finalize(b):
                o_sb = osb_pool.tile([1, D_ENC], f32, tag="osb")
                nc.vector.tensor_scalar_mul(o_sb[:], ctxz[b][:, 0:D_ENC],
                                            rz[b][:])
                nc.sync.dma_start(outd[b:b + 1, :], o_sb[:])

            for step in range(BPC + 1):
                if step < BPC:
                    S[step] = s_pool.tile([NCH, TC], f32, tag="srows",
                                          name=f"S{step}")
                for i in range(NCH):
                    if step < BPC:
                        emit_A_chunk(step, i)
                    if step >= 1:
                        emit_B_group(step - 1, 2 * i)
                        emit_B_group(step - 1, 2 * i + 1)
                if step < BPC:
                    emit_A_epilogue(step)
                if step >= 1:
                    emit_B_finalize(step - 1)

    nc.finalize()
    _PROGRAM_CACHE["nc"] = nc
    return nc


def _prep_inputs(encoder_out, hidden_state_h, hidden_state_c,
                 w1, b1, w2, b2, w3, b3, wv, bv):
    """Host-side sharding + layout prep. Returns per-core input maps."""
    enc = np.asarray(encoder_out, dtype=np.float32)
    # per-batch bias vector: b1 + h@w2 + b2 + c@w3 + b3  (tiny, exact f32)
    vb = (np.asarray(b1, np.float32)
          + np.asarray(hidden_state_h, np.float32) @ np.asarray(w2, np.float32)
          + np.asarray(b2, np.float32)
          + np.asarray(hidden_state_c, np.float32) @ np.asarray(w3, np.float32)
          + np.asarray(b3, np.float32))                        # [B, D_ATT]
    # bv shifts every score equally -> cancels in softmax; dropped.

    w1_h = np.ascontiguousarray(
        np.asarray(w1, np.float32).reshape(KD, P, NJ, P).transpose(1, 0, 2, 3)
    ).astype(BF16)                                             # [128,2,4,128]
    wv_h = np.ascontiguousarray(
        np.asarray(wv, np.float32).reshape(NJ, P).T).astype(BF16)  # [128,4]
    ident_h = np.eye(NCH, dtype=np.float32)

    in_maps = []
    for c in range(N_CORES):
        sl = slice(c * BPC, (c + 1) * BPC)
        enc_c = enc[sl]                                        # [4, T, 256]
        encT_c = np.ascontiguousarray(enc_c.transpose(0, 2, 1)).astype(BF16)
        encN_c = np.ascontiguousarray(enc_c).astype(BF16)
        vbt_c = np.ascontiguousarray(
            vb[sl].reshape(BPC, NJ, P).transpose(2, 0, 1).reshape(P, BPC * NJ)
        ).astype(np.float32)
        in_maps.append({
            "encT": encT_c,
            "encN": encN_c,
            "w1t": w1_h,
            "wvt": wv_h,
            "vbt": vbt_c,
            "ident": ident_h,
        })
    return in_maps


def kernel(**inputs):
    nc = _build_program()
    in_maps = _prep_inputs(**inputs)
    res = run_bass_kernel_spmd(nc, in_maps, list(range(N_CORES)))
    out = np.concatenate([res.results[c]["out"] for c in range(N_CORES)],
                         axis=0)
    return out.astype(np.float32)


if __name__ == "__main__":
    rng = np.random.default_rng(0)
    ins = {
        "encoder_out": rng.standard_normal((B, T, D_ENC), dtype=np.float32),
        "hidden_state_h": rng.standard_normal((B, D_HID), dtype=np.float32),
        "hidden_state_c": rng.standard_normal((B, D_HID), dtype=np.float32),
        "w1": (rng.standard_normal((D_ENC, D_ATT), dtype=np.float32)
               / np.sqrt(D_ENC)),
        "b1": np.zeros(D_ATT, np.float32),
        "w2": (rng.standard_normal((D_HID, D_ATT), dtype=np.float32)
               / np.sqrt(D_HID)),
        "b2": np.zeros(D_ATT, np.float32),
        "w3": (rng.standard_normal((D_HID, D_ATT), dtype=np.float32)
               / np.sqrt(D_HID)),
        "b3": np.zeros(D_ATT, np.float32),
        "wv": (rng.standard_normal((D_ATT, 1), dtype=np.float32)
               / np.sqrt(D_ATT)),
        "bv": np.zeros(1, np.float32),
    }
    got = kernel(**ins)
    print("kernel output:", got.shape, got.dtype)
